# revision 6
# baseline (speedup 1.0000x reference)
"""Trainium2 Bass kernel for nn_BridgeTowerBlock (dense transformer block).

Math notes (vs reference):
  - Attention has seq_len==1, so softmax over the singleton axis is exactly 1.0
    and attention output == relu(values @ wv + bv) + queries. The q/k
    projections are dead compute and are skipped (bit-identical result).
  - Per modality: o = x@Wb+bb ; t1 = relu(o@Wvs+bvs) + o + x ; m = LN1(t1)
    t2 = relu(m_other@Wvc+bvc) + 2*m ; n = LN3(t2)
    t3 = relu(n@W1+b1)@W2 + b2 + n ; out = LN5(t3)

Layout: activations are kept feature-on-partition ("d-on-p", i.e. transposed)
so every matmul consumes them directly; weights ride as the stationary
operand in natural [d_in, d_out] layout. LayerNorm stats are computed with
ones-vector matmuls (partition reduction on the PE); per-token mean/rstd are
broadcast across partitions with K=1 matmuls. The final LN runs in token
space (bn_stats) because the output must be transposed back for the store
anyway. Matmul inputs are bf16 (1 cycle/row on the PE vs 4 for fp32);
broadcasts use float32r.

Sharding: pure data parallel, 65536 tokens split across 8 cores (8192 each),
weights replicated.
"""

import os
import sys

for _p in ("/opt/trn_rl_repo", "/root/.axon_site/_ro/trn_rl_repo"):
    if os.path.isdir(_p) and _p not in sys.path:
        sys.path.insert(0, _p)

from contextlib import ExitStack

import ml_dtypes
import numpy as np

import concourse.bass as bass
import concourse.tile as tile
from concourse import bacc, mybir
from concourse.bass_utils import run_bass_kernel_spmd

F32 = mybir.dt.float32
BF16 = mybir.dt.bfloat16
F32R = mybir.dt.float32r
AF = mybir.ActivationFunctionType
OP = mybir.AluOpType

H = 512
FFN = 1024
P = 128
T = 512          # tokens per tile
N_CORES = 8
N_TOTAL = 65536
EPS = 1e-5

_CACHE = {}


def _emit_tile(nc, pools, W, i, x_dram, out_dram, affine14, affine56):
    """Emit one 2*T-token tile (T tokens x 2 modalities)."""
    sb, ps_mm, ps_tp, ps_st, tiny, tiny5 = (
        pools["sb"], pools["ps_mm"], pools["ps_tp"], pools["ps_st"],
        pools["tiny"], pools["tiny5"],
    )
    tok0 = i * T
    C1 = 1.0 / float(H)

    def mk(tag, shape, dt, bufs=1):
        return sb.tile(shape, dt, tag=tag, name=tag, bufs=bufs)

    xT = [None, None]
    o = [None, None]
    t1 = [None, None]
    m_ = [None, None]
    n_ = [None, None]
    nnat = [None, None]
    h_ = [None, None]
    t3 = [None, None]
    osb = [None, None]

    def ln_dp(m, t_in, gb):
        """d-on-p layernorm of t_in [P,4,T] bf16 -> new [P,4,T] bf16."""
        S = ps_st.tile([1, T], F32, tag="st")
        Q = ps_st.tile([1, T], F32, tag="st")
        for c in range(4):
            nc.tensor.matmul(S[:], W["ones_stat"][:, 0:1], t_in[:, c, :],
                             start=(c == 0), stop=(c == 3))
        for c in range(4):
            sq = sb.tile([P, T], BF16, tag=f"sq{m}", name=f"sq{m}", bufs=2)
            nc.vector.tensor_tensor(sq[:], t_in[:, c, :], t_in[:, c, :], OP.mult)
            nc.tensor.matmul(Q[:], W["ones_stat"][:, 0:1], sq[:],
                             start=(c == 0), stop=(c == 3))
        musq = tiny.tile([1, T], F32, tag="lnt")
        nc.scalar.activation(musq[:], S[:], AF.Square, scale=C1)
        var = tiny.tile([1, T], F32, tag="lnt")
        nc.vector.scalar_tensor_tensor(out=var[:], in0=Q[:], scalar=C1, in1=musq[:],
                                       op0=OP.mult, op1=OP.subtract)
        sd = tiny.tile([1, T], F32, tag="lnt")
        nc.scalar.activation(sd[:], var[:], AF.Sqrt, bias=W["eps1"][:, 0:1])
        rstd = tiny.tile([1, T], F32R, tag="lnt")
        with nc.allow_low_precision(reason="f32r rstd feeds f32r broadcast matmul"):
            nc.vector.reciprocal(rstd[:], sd[:])
        mr = tiny.tile([1, T], F32R, tag="lnt")
        nc.vector.scalar_tensor_tensor(out=mr[:], in0=S[:], scalar=C1, in1=rstd[:].bitcast(F32),
                                       op0=OP.mult, op1=OP.mult)
        rb = ps_mm.tile([P, T], F32, tag="mm")
        nc.tensor.matmul(rb[:], W["onesrow_r"][:, :], rstd[:], start=True, stop=True)
        mb = ps_mm.tile([P, T], F32, tag="mm")
        nc.tensor.matmul(mb[:], W["onesrow_r"][:, :], mr[:], start=True, stop=True)
        nb = mk(f"nb{m}", [P, 2, T], BF16)
        nc.scalar.activation(nb[:, 0, :], rb[:], AF.Copy)
        nc.scalar.activation(nb[:, 1, :], mb[:], AF.Copy)
        t_out = mk(f"mn{m}", [P, 4, T], BF16)
        for c in range(4):
            nc.vector.tensor_tensor(t_out[:, c, :], t_in[:, c, :], nb[:, 0, :], OP.mult)
            nc.vector.tensor_tensor(t_out[:, c, :], t_out[:, c, :], nb[:, 1, :], OP.subtract)
            if gb is not None:
                nc.vector.tensor_scalar(out=t_out[:, c, :], in0=t_out[:, c, :],
                                        scalar1=gb[0][:, c:c + 1], scalar2=gb[1][:, c:c + 1],
                                        op0=OP.mult, op1=OP.add)
        return t_out

    # ---- A: load + transpose x ----
    for m in (0, 1):
        xnat = sb.tile([P, 4, T], F32, tag=f"xnat{m}", bufs=1)
        src = x_dram[m][tok0:tok0 + T, :].rearrange("(b p) d -> p b d", p=P)
        nc.sync.dma_start(xnat[:], src)
        xT[m] = mk(f"xT{m}", [P, 4, T], BF16)
        for ko in range(4):
            pt = ps_mm.tile([P, T], F32, tag="mm")
            for b in range(4):
                nc.tensor.transpose(pt[:, b * P:(b + 1) * P],
                                    xnat[:, b, ko * P:(ko + 1) * P], W["ident32"][:])
            nc.scalar.activation(xT[m][:, ko, :], pt[:], AF.Copy)

    # ---- B: bridge ----
    for m in (0, 1):
        o[m] = mk(f"o{m}", [P, 4, T], BF16)
        for mo in range(4):
            pmm = ps_mm.tile([P, T], F32, tag="mm")
            for ko in range(4):
                nc.tensor.matmul(pmm[:], W[f"wb{m}"][:, ko, mo * P:(mo + 1) * P],
                                 xT[m][:, ko, :], start=(ko == 0), stop=(ko == 3))
            nc.scalar.activation(o[m][:, mo, :], pmm[:], AF.Identity,
                                 bias=W[f"bb{m}"][:, mo:mo + 1])

    # ---- C: self-attn value proj + residual ----
    for m in (0, 1):
        r = mk(f"r{m}", [P, 4, T], BF16)
        for mo in range(4):
            pmm = ps_mm.tile([P, T], F32, tag="mm")
            for ko in range(4):
                nc.tensor.matmul(pmm[:], W[f"wvs{m}"][:, ko, mo * P:(mo + 1) * P],
                                 o[m][:, ko, :], start=(ko == 0), stop=(ko == 3))
            nc.scalar.activation(r[:, mo, :], pmm[:],
                                 AF.Relu, bias=W[f"bvs{m}"][:, mo:mo + 1])
        t1[m] = mk(f"t{m}", [P, 4, T], BF16)
        for c in range(4):
            nc.gpsimd.tensor_tensor(t1[m][:, c, :], r[:, c, :], o[m][:, c, :], OP.add)
            nc.gpsimd.tensor_tensor(t1[m][:, c, :], t1[m][:, c, :], xT[m][:, c, :], OP.add)

    # ---- D: LN1/LN2 ----
    for m in (0, 1):
        m_[m] = ln_dp(m, t1[m], (W[f"g{m}_a"], W[f"b{m}_a"]) if affine14 else None)

    # ---- E: cross-attn value proj + residual ----
    for m in (0, 1):
        r2 = mk(f"r{m}", [P, 4, T], BF16)
        src_m = m_[1 - m]
        for mo in range(4):
            pmm = ps_mm.tile([P, T], F32, tag="mm")
            for ko in range(4):
                nc.tensor.matmul(pmm[:], W[f"wvc{m}"][:, ko, mo * P:(mo + 1) * P],
                                 src_m[:, ko, :], start=(ko == 0), stop=(ko == 3))
            nc.scalar.activation(r2[:, mo, :], pmm[:], AF.Relu,
                                 bias=W[f"bvc{m}"][:, mo:mo + 1])
        t2 = mk(f"t{m}", [P, 4, T], BF16)
        for c in range(4):
            nc.gpsimd.tensor_tensor(t2[:, c, :], r2[:, c, :], m_[m][:, c, :], OP.add)
            nc.gpsimd.tensor_tensor(t2[:, c, :], t2[:, c, :], m_[m][:, c, :], OP.add)
        t1[m] = t2  # reuse slot name for LN3 input

    # ---- F: LN3/LN4 ----
    for m in (0, 1):
        n_[m] = ln_dp(m, t1[m], (W[f"g{m}_b"], W[f"b{m}_b"]) if affine14 else None)

    # ---- G: ffn1 + transpose n ----
    for m in (0, 1):
        h_[m] = mk(f"h{m}", [P, 8, T], BF16)
        for mo in range(8):
            pmm = ps_mm.tile([P, T], F32, tag="mm")
            for ko in range(4):
                nc.tensor.matmul(pmm[:], W[f"w1{m}"][:, ko, mo * P:(mo + 1) * P],
                                 n_[m][:, ko, :], start=(ko == 0), stop=(ko == 3))
            nc.scalar.activation(h_[m][:, mo, :], pmm[:], AF.Relu,
                                 bias=W[f"b1{m}"][:, mo:mo + 1])
        nnat[m] = mk(f"nnat{m}", [P, 4, T], BF16)
        for b in range(4):
            pt = ps_tp.tile([P, T], BF16, tag="tp")
            for ko in range(4):
                nc.tensor.transpose(pt[:, ko * P:(ko + 1) * P],
                                    n_[m][:, ko, b * P:(b + 1) * P], W["ident16"][:])
            nc.scalar.activation(nnat[m][:, b, :], pt[:], AF.Copy)

    # ---- H: ffn2 (activations stationary) + residual -> token space ----
    for m in (0, 1):
        t3[m] = sb.tile([P, 4, T], F32, tag=f"t3_{m}", name=f"t3_{m}", bufs=1)
        for b in range(4):
            pmm = ps_mm.tile([P, T], F32, tag="mm")
            for ko in range(8):
                nc.tensor.matmul(pmm[:], h_[m][:, ko, b * P:(b + 1) * P],
                                 W[f"w2{m}"][:, ko, :], start=(ko == 0), stop=(ko == 7))
            nc.vector.scalar_tensor_tensor(out=t3[m][:, b, :], in0=pmm[:], scalar=1.0,
                                           in1=nnat[m][:, b, :], op0=OP.mult, op1=OP.add)
            nc.gpsimd.tensor_tensor(t3[m][:, b, :], t3[m][:, b, :], W[f"b2b{m}"][:], OP.add)

    # ---- I: LN5/LN6 in token space + store ----
    for m in (0, 1):
        for b in range(4):
            bn6 = tiny5.tile([P, 6], F32, tag="bn6")
            nc.vector.bn_stats(bn6[:], t3[m][:, b, :])
            mv = tiny5.tile([P, 2], F32, tag="mv")
            nc.vector.bn_aggr(mv[:], bn6[:])
            sd5 = tiny5.tile([P, 1], F32, tag="sd5")
            nc.scalar.activation(sd5[:], mv[:, 1:2], AF.Sqrt, bias=W["eps128"][:, 0:1])
            rstd5 = tiny5.tile([P, 1], F32, tag="rstd5")
            nc.vector.reciprocal(rstd5[:], sd5[:])
            ob = sb.tile([P, T], F32, tag=f"ob{m}", name=f"ob{m}", bufs=2)
            nc.vector.tensor_scalar(out=ob[:], in0=t3[m][:, b, :],
                                    scalar1=mv[:, 0:1], scalar2=rstd5[:],
                                    op0=OP.subtract, op1=OP.mult)
            if affine56:
                nc.gpsimd.tensor_tensor(ob[:], ob[:], W[f"g5b{m}"][:], OP.mult)
                nc.gpsimd.tensor_tensor(ob[:], ob[:], W[f"b5b{m}"][:], OP.add)
            dst = out_dram[m][tok0 + b * P:tok0 + (b + 1) * P, :]
            nc.sync.dma_start(dst, ob[:])


def _build(n_tok, affine14, affine56):
    nc = bacc.Bacc("TRN2", target_bir_lowering=False, debug=False,
                   num_devices=N_CORES)
    ntiles = n_tok // T

    x1 = nc.declare_dram_parameter("x1", [n_tok, H], F32, isOutput=False)
    x2 = nc.declare_dram_parameter("x2", [n_tok, H], F32, isOutput=False)
    out1 = nc.declare_dram_parameter("out1", [n_tok, H], F32, isOutput=True)
    out2 = nc.declare_dram_parameter("out2", [n_tok, H], F32, isOutput=True)

    wd = {}

    def dparam(name, shape, dt):
        wd[name] = nc.declare_dram_parameter(name, shape, dt, isOutput=False)

    for m in (0, 1):
        dparam(f"wb{m}", [P, 4, H], BF16)
        dparam(f"wvs{m}", [P, 4, H], BF16)
        dparam(f"wvc{m}", [P, 4, H], BF16)
        dparam(f"w1{m}", [P, 4, FFN], BF16)
        dparam(f"w2{m}", [P, 8, H], BF16)
        dparam(f"bb{m}", [P, 4], F32)
        dparam(f"bvs{m}", [P, 4], F32)
        dparam(f"bvc{m}", [P, 4], F32)
        dparam(f"b1{m}", [P, 8], F32)
        dparam(f"b2b{m}", [P, H], F32)
        if affine14:
            dparam(f"g{m}_a", [P, 4], F32)
            dparam(f"b{m}_a", [P, 4], F32)
            dparam(f"g{m}_b", [P, 4], F32)
            dparam(f"b{m}_b", [P, 4], F32)
        if affine56:
            dparam(f"g5b{m}", [P, H], F32)
            dparam(f"b5b{m}", [P, H], F32)
    dparam("ident32", [P, P], F32)
    dparam("ident16", [P, P], BF16)
    dparam("ones_stat", [P, 4], BF16)
    dparam("onesrow_r", [1, P], F32R)
    dparam("eps1", [1, 4], F32)
    dparam("eps128", [P, 4], F32)

    with tile.TileContext(nc) as tc, ExitStack() as ctx:
        sb = ctx.enter_context(tc.tile_pool(name="sb", bufs=1))
        consts = ctx.enter_context(tc.tile_pool(name="consts", bufs=1))
        ps_mm = ctx.enter_context(tc.tile_pool(name="ps_mm", bufs=3, space="PSUM"))
        ps_tp = ctx.enter_context(tc.tile_pool(name="ps_tp", bufs=2, space="PSUM"))
        ps_st = ctx.enter_context(tc.tile_pool(name="ps_st", bufs=2, space="PSUM"))
        tiny = ctx.enter_context(tc.tile_pool(name="tiny", bufs=4))
        tiny5 = ctx.enter_context(tc.tile_pool(name="tiny5", bufs=4))

        W = {}
        for name, dram in wd.items():
            t = consts.tile(list(dram.shape), dram.dtype, tag=name)
            nc.sync.dma_start(t[:], dram[:])
            W[name] = t

        pools = {"sb": sb, "ps_mm": ps_mm, "ps_tp": ps_tp, "ps_st": ps_st,
                 "tiny": tiny, "tiny5": tiny5}
        # pool with extra buffering for the fp32 [P,4,T] tiles
        for i in range(ntiles):
            _emit_tile(nc, pools, W, i, (x1, x2), (out1, out2), affine14, affine56)

    nc.compile()
    return nc


def _host_prep(params):
    """Flatten params into the per-core replicated input map."""
    def npf(a):
        return np.asarray(a, dtype=np.float32)

    def wmat(wkey, dout):
        w = npf(wkey)
        kin = w.shape[0]
        return np.ascontiguousarray(
            w.reshape(kin // P, P, dout).transpose(1, 0, 2)).astype(ml_dtypes.bfloat16)

    def bcol(b):
        b = npf(b)
        return np.ascontiguousarray(b.reshape(-1, P).T)

    mp = {}
    ln_names = [("ln1", "ln3", "ln5"), ("ln2", "ln4", "ln6")]
    affine14 = False
    affine56 = False
    for m in (0, 1):
        sfx = str(m + 1)
        mp[f"wb{m}"] = wmat(params["bridge" + sfx]["w"], H)
        mp[f"bb{m}"] = bcol(params["bridge" + sfx]["b"])
        mp[f"wvs{m}"] = wmat(params["sa" + sfx]["wv"], H)
        mp[f"bvs{m}"] = bcol(params["sa" + sfx]["bv"])
        mp[f"wvc{m}"] = wmat(params["ca" + sfx]["wv"], H)
        mp[f"bvc{m}"] = bcol(params["ca" + sfx]["bv"])
        mp[f"w1{m}"] = wmat(params["ffn" + sfx]["w1"], FFN)
        mp[f"b1{m}"] = bcol(params["ffn" + sfx]["b1"])
        mp[f"w2{m}"] = wmat(params["ffn" + sfx]["w2"], H)
        mp[f"b2b{m}"] = np.ascontiguousarray(
            np.broadcast_to(npf(params["ffn" + sfx]["b2"]), (P, H)))
        la, lb, lc = ln_names[m]
        for lk, a_sfx in ((la, "_a"), (lb, "_b")):
            g = npf(params[lk]["g"]); b = npf(params[lk]["b"])
            if not (np.all(g == 1.0) and np.all(b == 0.0)):
                affine14 = True
            mp[f"g{m}{a_sfx}"] = bcol(g)
            mp[f"b{m}{a_sfx}"] = bcol(b)
        g = npf(params[lc]["g"]); b = npf(params[lc]["b"])
        if not (np.all(g == 1.0) and np.all(b == 0.0)):
            affine56 = True
        mp[f"g5b{m}"] = np.ascontiguousarray(np.broadcast_to(g, (P, H)))
        mp[f"b5b{m}"] = np.ascontiguousarray(np.broadcast_to(b, (P, H)))
    if not affine14:
        for m in (0, 1):
            for k in (f"g{m}_a", f"b{m}_a", f"g{m}_b", f"b{m}_b"):
                del mp[k]
    if not affine56:
        for m in (0, 1):
            del mp[f"g5b{m}"]
            del mp[f"b5b{m}"]
    mp["ident32"] = np.eye(P, dtype=np.float32)
    mp["ident16"] = np.eye(P, dtype=ml_dtypes.bfloat16)
    ones_stat = np.zeros((P, 4), ml_dtypes.bfloat16); ones_stat[:, 0] = 1.0
    mp["ones_stat"] = ones_stat
    mp["onesrow_r"] = np.ones((1, P), np.float32)
    mp["eps1"] = np.full((1, 4), EPS, np.float32)
    mp["eps128"] = np.full((P, 4), EPS, np.float32)
    return mp, affine14, affine56


def _get_program(n_tok, affine14, affine56):
    key = (n_tok, affine14, affine56)
    if key not in _CACHE:
        _CACHE[key] = _build(n_tok, affine14, affine56)
    return _CACHE[key]


def run(modality_1, modality_2, params, n_cores=N_CORES):
    m1 = np.ascontiguousarray(np.asarray(modality_1, dtype=np.float32))
    m2 = np.ascontiguousarray(np.asarray(modality_2, dtype=np.float32))
    n_total = m1.shape[0]
    n_tok = n_total // n_cores
    assert n_tok % T == 0, f"tokens per core ({n_tok}) must be a multiple of {T}"
    mp, affine14, affine56 = _host_prep(params)
    nc = _get_program(n_tok, affine14, affine56)
    in_maps = []
    for c in range(n_cores):
        d = dict(mp)
        d["x1"] = m1[c * n_tok:(c + 1) * n_tok]
        d["x2"] = m2[c * n_tok:(c + 1) * n_tok]
        in_maps.append(d)
    res = run_bass_kernel_spmd(nc, in_maps, list(range(n_cores)))
    o1 = np.concatenate([res.results[c]["out1"] for c in range(n_cores)], axis=0)
    o2 = np.concatenate([res.results[c]["out2"] for c in range(n_cores)], axis=0)
    return o1, o2


def kernel(modality_1, modality_2, params):
    return run(modality_1, modality_2, params)


# revision 7
# speedup vs baseline: 991.4086x; 991.4086x over previous
"""Trainium2 Bass kernel for nn_BridgeTowerBlock (dense transformer block).

Math notes (vs reference):
  - Attention has seq_len==1, so softmax over the singleton axis is exactly 1.0
    and attention output == relu(values @ wv + bv) + queries. The q/k
    projections are dead compute and are skipped (bit-identical result).
  - Per modality: o = x@Wb+bb ; t1 = relu(o@Wvs+bvs) + o + x ; m = LN1(t1)
    t2 = relu(m_other@Wvc+bvc) + 2*m ; n = LN3(t2)
    t3 = relu(n@W1+b1)@W2 + b2 + n ; out = LN5(t3)

Layout: activations are kept feature-on-partition ("d-on-p", i.e. transposed)
so every matmul consumes them directly; weights ride as the stationary
operand in natural [d_in, d_out] layout. LayerNorm stats are computed with
ones-vector matmuls (partition reduction on the PE); per-token mean/rstd are
broadcast across partitions with K=1 float32r matmuls. The final LN runs in
token space (bn_stats) because the output must be transposed back for the
store anyway.

Precision: matmul operands are bf16 (1 cycle/row on the PE vs 4 for fp32),
but the residual stream (o, t1, t2, m, n, t3) is kept in fp32 — residual
rounding dominates the error otherwise (measured 6x difference).

Sharding: pure data parallel, 65536 tokens split across 8 cores (8192 each),
weights replicated.
"""

import os
import sys

for _p in ("/opt/trn_rl_repo", "/root/.axon_site/_ro/trn_rl_repo"):
    if os.path.isdir(_p) and _p not in sys.path:
        sys.path.insert(0, _p)

from contextlib import ExitStack

import ml_dtypes
import numpy as np

import concourse.bass as bass
import concourse.tile as tile
from concourse import bacc, mybir
from concourse.bass_utils import run_bass_kernel_spmd

F32 = mybir.dt.float32
BF16 = mybir.dt.bfloat16
F32R = mybir.dt.float32r
AF = mybir.ActivationFunctionType
OP = mybir.AluOpType

H = 512
FFN = 1024
P = 128
T = 256          # tokens per tile
B = T // P       # 128-token blocks per tile
N_CORES = 8
EPS = 1e-5

_CACHE = {}


def _emit_tile(nc, pools, W, i, x_dram, out_dram, affine14, affine56):
    """Emit one tile: T tokens x 2 modalities."""
    sb, ps_mm, ps_st, tiny, tiny5 = (
        pools["sb"], pools["ps_mm"], pools["ps_st"], pools["tiny"], pools["tiny5"],
    )
    tok0 = i * T
    C1 = 1.0 / float(H)

    def mk(tag, shape, dt, bufs=1):
        return sb.tile(shape, dt, tag=tag, name=tag, bufs=bufs)

    xT32 = [None, None]
    xT16 = [None, None]
    o32 = [None, None]
    o16 = [None, None]
    t32 = [None, None]
    m32 = [None, None]
    m16 = [None, None]
    n32 = [None, None]
    n16 = [None, None]
    nnat = [None, None]
    h_ = [None, None]
    t3 = [None, None]

    def ln_dp(m, t_in, gb):
        """d-on-p layernorm of t_in [P,4,T] f32 -> (f32, bf16) outputs."""
        S = ps_st.tile([1, T], F32, tag="st")
        Q = ps_st.tile([1, T], F32, tag="st")
        for c in range(4):
            t16 = sb.tile([P, T], BF16, tag=f"t16c{m}", name=f"t16c{m}", bufs=2)
            nc.scalar.activation(t16[:], t_in[:, c, :], AF.Copy)
            nc.tensor.matmul(S[:], W["ones_stat"][:, 0:1], t16[:],
                             start=(c == 0), stop=(c == 3))
            sq = sb.tile([P, T], BF16, tag=f"sqc{m}", name=f"sqc{m}", bufs=2)
            nc.vector.tensor_tensor(sq[:], t16[:], t16[:], OP.mult)
            nc.tensor.matmul(Q[:], W["ones_stat"][:, 0:1], sq[:],
                             start=(c == 0), stop=(c == 3))
        musq = tiny.tile([1, T], F32, tag="lnt")
        nc.scalar.activation(musq[:], S[:], AF.Square, scale=C1)
        var = tiny.tile([1, T], F32, tag="lnt")
        nc.vector.scalar_tensor_tensor(out=var[:], in0=Q[:], scalar=C1, in1=musq[:],
                                       op0=OP.mult, op1=OP.subtract)
        sd = tiny.tile([1, T], F32, tag="lnt")
        nc.scalar.activation(sd[:], var[:], AF.Sqrt, bias=W["eps1"][:, 0:1])
        rstd = tiny.tile([1, T], F32R, tag="lnt")
        with nc.allow_low_precision(reason="f32r rstd feeds f32r broadcast matmul"):
            nc.vector.reciprocal(rstd[:], sd[:])
        mr = tiny.tile([1, T], F32R, tag="lnt")
        nc.vector.scalar_tensor_tensor(out=mr[:], in0=S[:], scalar=C1,
                                       in1=rstd[:].bitcast(F32),
                                       op0=OP.mult, op1=OP.mult)
        rb = ps_mm.tile([P, T], F32, tag="mm")
        nc.tensor.matmul(rb[:], W["onesrow_r"][:, :], rstd[:], start=True, stop=True)
        mb = ps_mm.tile([P, T], F32, tag="mm")
        nc.tensor.matmul(mb[:], W["onesrow_r"][:, :], mr[:], start=True, stop=True)
        nb = mk(f"nb{m}", [P, 2, T], F32)
        nc.scalar.activation(nb[:, 0, :], rb[:], AF.Copy)
        nc.scalar.activation(nb[:, 1, :], mb[:], AF.Copy)
        z32 = mk(f"mn32_{m}", [P, 4, T], F32)
        z16 = mk(f"mn16_{m}", [P, 4, T], BF16)
        for c in range(4):
            nc.vector.tensor_tensor(z32[:, c, :], t_in[:, c, :], nb[:, 0, :], OP.mult)
            nc.vector.tensor_tensor(z32[:, c, :], z32[:, c, :], nb[:, 1, :], OP.subtract)
            if gb is not None:
                nc.vector.tensor_scalar(out=z32[:, c, :], in0=z32[:, c, :],
                                        scalar1=gb[0][:, c:c + 1], scalar2=gb[1][:, c:c + 1],
                                        op0=OP.mult, op1=OP.add)
            nc.vector.tensor_copy(z16[:, c, :], z32[:, c, :])
        return z32, z16

    # ---- A: load + transpose x (keep f32 and bf16 forms) ----
    for m in (0, 1):
        xnat = sb.tile([P, B, H], F32, tag=f"xnat{m}", name=f"xnat{m}", bufs=2)
        src = x_dram[m][tok0:tok0 + T, :].rearrange("(b p) d -> p b d", p=P)
        nc.sync.dma_start(xnat[:], src)
        xT32[m] = mk(f"xT32_{m}", [P, 4, T], F32)
        xT16[m] = mk(f"xT16_{m}", [P, 4, T], BF16)
        for ko in range(4):
            pt = ps_mm.tile([P, T], F32, tag="mm")
            for b in range(B):
                nc.tensor.transpose(pt[:, b * P:(b + 1) * P],
                                    xnat[:, b, ko * P:(ko + 1) * P], W["ident32"][:])
            nc.scalar.activation(xT32[m][:, ko, :], pt[:], AF.Copy)
            nc.scalar.activation(xT16[m][:, ko, :], pt[:], AF.Copy)

    # ---- B: bridge ----
    for m in (0, 1):
        o32[m] = mk(f"o32_{m}", [P, 4, T], F32)
        o16[m] = mk(f"o16_{m}", [P, 4, T], BF16)
        for mo in range(4):
            pmm = ps_mm.tile([P, T], F32, tag="mm")
            for ko in range(4):
                nc.tensor.matmul(pmm[:], W[f"wb{m}"][:, ko, mo * P:(mo + 1) * P],
                                 xT16[m][:, ko, :], start=(ko == 0), stop=(ko == 3))
            nc.scalar.activation(o32[m][:, mo, :], pmm[:], AF.Identity,
                                 bias=W[f"bb{m}"][:, mo:mo + 1])
            nc.scalar.activation(o16[m][:, mo, :], pmm[:], AF.Identity,
                                 bias=W[f"bb{m}"][:, mo:mo + 1])

    # ---- C: self-attn value proj + residual: t1 = relu(..) + o + x ----
    for m in (0, 1):
        t32[m] = mk(f"t32_{m}", [P, 4, T], F32)
        for mo in range(4):
            pmm = ps_mm.tile([P, T], F32, tag="mm")
            for ko in range(4):
                nc.tensor.matmul(pmm[:], W[f"wvs{m}"][:, ko, mo * P:(mo + 1) * P],
                                 o16[m][:, ko, :], start=(ko == 0), stop=(ko == 3))
            nc.scalar.activation(t32[m][:, mo, :], pmm[:], AF.Relu,
                                 bias=W[f"bvs{m}"][:, mo:mo + 1])
        for c in range(4):
            nc.vector.scalar_tensor_tensor(out=t32[m][:, c, :], in0=t32[m][:, c, :],
                                           scalar=1.0, in1=o32[m][:, c, :],
                                           op0=OP.mult, op1=OP.add)
            nc.gpsimd.tensor_tensor(t32[m][:, c, :], t32[m][:, c, :],
                                    xT32[m][:, c, :], OP.add)

    # ---- D: LN1/LN2 ----
    for m in (0, 1):
        m32[m], m16[m] = ln_dp(m, t32[m],
                               (W[f"g{m}_a"], W[f"b{m}_a"]) if affine14 else None)

    # ---- E: cross-attn value proj + residual: t2 = relu(..) + 2*m ----
    for m in (0, 1):
        t2 = mk(f"t32_{m}", [P, 4, T], F32)
        for mo in range(4):
            pmm = ps_mm.tile([P, T], F32, tag="mm")
            for ko in range(4):
                nc.tensor.matmul(pmm[:], W[f"wvc{m}"][:, ko, mo * P:(mo + 1) * P],
                                 m16[1 - m][:, ko, :], start=(ko == 0), stop=(ko == 3))
            nc.scalar.activation(t2[:, mo, :], pmm[:], AF.Relu,
                                 bias=W[f"bvc{m}"][:, mo:mo + 1])
        for c in range(4):
            nc.vector.scalar_tensor_tensor(out=t2[:, c, :], in0=m32[m][:, c, :],
                                           scalar=2.0, in1=t2[:, c, :],
                                           op0=OP.mult, op1=OP.add)
        t32[m] = t2

    # ---- F: LN3/LN4 ----
    for m in (0, 1):
        n32[m], n16[m] = ln_dp(m, t32[m],
                               (W[f"g{m}_b"], W[f"b{m}_b"]) if affine14 else None)

    # ---- G: ffn1 + transpose n back to token space (f32) ----
    for m in (0, 1):
        h_[m] = mk(f"h{m}", [P, 8, T], BF16)
        for mo in range(8):
            pmm = ps_mm.tile([P, T], F32, tag="mm")
            for ko in range(4):
                nc.tensor.matmul(pmm[:], W[f"w1{m}"][:, ko, mo * P:(mo + 1) * P],
                                 n16[m][:, ko, :], start=(ko == 0), stop=(ko == 3))
            nc.scalar.activation(h_[m][:, mo, :], pmm[:], AF.Relu,
                                 bias=W[f"b1{m}"][:, mo:mo + 1])
        nnat[m] = mk(f"nnat{m}", [P, B, H], F32)
        for b in range(B):
            pt = ps_mm.tile([P, H], F32, tag="mm5")
            for ko in range(4):
                nc.tensor.transpose(pt[:, ko * P:(ko + 1) * P],
                                    n32[m][:, ko, b * P:(b + 1) * P], W["ident32"][:])
            nc.scalar.activation(nnat[m][:, b, :], pt[:], AF.Copy)

    # ---- H: ffn2 (h stationary) -> token space + residual ----
    for m in (0, 1):
        t3[m] = mk(f"t3_{m}", [P, B, H], F32)
        for b in range(B):
            pmm = ps_mm.tile([P, H], F32, tag="mm5")
            for ko in range(8):
                nc.tensor.matmul(pmm[:], h_[m][:, ko, b * P:(b + 1) * P],
                                 W[f"w2{m}"][:, ko, :], start=(ko == 0), stop=(ko == 7))
            nc.vector.scalar_tensor_tensor(out=t3[m][:, b, :], in0=pmm[:], scalar=1.0,
                                           in1=nnat[m][:, b, :], op0=OP.mult, op1=OP.add)
            nc.gpsimd.tensor_tensor(t3[m][:, b, :], t3[m][:, b, :],
                                    W[f"b2b{m}"][:], OP.add)

    # ---- I: LN5/LN6 in token space + store ----
    for m in (0, 1):
        for b in range(B):
            bn6 = tiny5.tile([P, 6], F32, tag="bn6")
            nc.vector.bn_stats(bn6[:], t3[m][:, b, :])
            mv = tiny5.tile([P, 2], F32, tag="mv")
            nc.vector.bn_aggr(mv[:], bn6[:])
            sd5 = tiny5.tile([P, 1], F32, tag="sd5")
            nc.scalar.activation(sd5[:], mv[:, 1:2], AF.Sqrt, bias=W["eps128"][:, 0:1])
            rstd5 = tiny5.tile([P, 1], F32, tag="rstd5")
            nc.vector.reciprocal(rstd5[:], sd5[:])
            ob = sb.tile([P, H], F32, tag=f"ob{m}", name=f"ob{m}", bufs=2)
            nc.vector.tensor_scalar(out=ob[:], in0=t3[m][:, b, :],
                                    scalar1=mv[:, 0:1], scalar2=rstd5[:],
                                    op0=OP.subtract, op1=OP.mult)
            if affine56:
                nc.gpsimd.tensor_tensor(ob[:], ob[:], W[f"g5b{m}"][:], OP.mult)
                nc.gpsimd.tensor_tensor(ob[:], ob[:], W[f"b5b{m}"][:], OP.add)
            dst = out_dram[m][tok0 + b * P:tok0 + (b + 1) * P, :]
            nc.sync.dma_start(dst, ob[:])


def _build(n_tok, affine14, affine56):
    nc = bacc.Bacc("TRN2", target_bir_lowering=False, debug=False,
                   num_devices=N_CORES)
    ntiles = n_tok // T

    x1 = nc.declare_dram_parameter("x1", [n_tok, H], F32, isOutput=False)
    x2 = nc.declare_dram_parameter("x2", [n_tok, H], F32, isOutput=False)
    out1 = nc.declare_dram_parameter("out1", [n_tok, H], F32, isOutput=True)
    out2 = nc.declare_dram_parameter("out2", [n_tok, H], F32, isOutput=True)

    wd = {}

    def dparam(name, shape, dt):
        wd[name] = nc.declare_dram_parameter(name, shape, dt, isOutput=False)

    for m in (0, 1):
        dparam(f"wb{m}", [P, 4, H], BF16)
        dparam(f"wvs{m}", [P, 4, H], BF16)
        dparam(f"wvc{m}", [P, 4, H], BF16)
        dparam(f"w1{m}", [P, 4, FFN], BF16)
        dparam(f"w2{m}", [P, 8, H], BF16)
        dparam(f"bb{m}", [P, 4], F32)
        dparam(f"bvs{m}", [P, 4], F32)
        dparam(f"bvc{m}", [P, 4], F32)
        dparam(f"b1{m}", [P, 8], F32)
        dparam(f"b2b{m}", [P, H], F32)
        if affine14:
            dparam(f"g{m}_a", [P, 4], F32)
            dparam(f"b{m}_a", [P, 4], F32)
            dparam(f"g{m}_b", [P, 4], F32)
            dparam(f"b{m}_b", [P, 4], F32)
        if affine56:
            dparam(f"g5b{m}", [P, H], F32)
            dparam(f"b5b{m}", [P, H], F32)
    dparam("ident32", [P, P], F32)
    dparam("ones_stat", [P, 4], BF16)
    dparam("onesrow_r", [1, P], F32R)
    dparam("eps1", [1, 4], F32)
    dparam("eps128", [P, 4], F32)

    with tile.TileContext(nc) as tc, ExitStack() as ctx:
        sb = ctx.enter_context(tc.tile_pool(name="sb", bufs=1))
        consts = ctx.enter_context(tc.tile_pool(name="consts", bufs=1))
        ps_mm = ctx.enter_context(tc.tile_pool(name="ps_mm", bufs=3, space="PSUM"))
        ps_st = ctx.enter_context(tc.tile_pool(name="ps_st", bufs=2, space="PSUM"))
        tiny = ctx.enter_context(tc.tile_pool(name="tiny", bufs=4))
        tiny5 = ctx.enter_context(tc.tile_pool(name="tiny5", bufs=4))

        W = {}
        for name, dram in wd.items():
            t = consts.tile(list(dram.shape), dram.dtype, tag=name)
            nc.sync.dma_start(t[:], dram[:])
            W[name] = t

        pools = {"sb": sb, "ps_mm": ps_mm, "ps_st": ps_st,
                 "tiny": tiny, "tiny5": tiny5}
        for i in range(ntiles):
            _emit_tile(nc, pools, W, i, (x1, x2), (out1, out2), affine14, affine56)

    nc.compile()
    return nc


def _host_prep(params):
    """Flatten params into the per-core replicated input map."""
    def npf(a):
        return np.asarray(a, dtype=np.float32)

    def wmat(wkey, dout):
        w = npf(wkey)
        kin = w.shape[0]
        return np.ascontiguousarray(
            w.reshape(kin // P, P, dout).transpose(1, 0, 2)).astype(ml_dtypes.bfloat16)

    def bcol(b):
        b = npf(b)
        return np.ascontiguousarray(b.reshape(-1, P).T)

    mp = {}
    ln_names = [("ln1", "ln3", "ln5"), ("ln2", "ln4", "ln6")]
    affine14 = False
    affine56 = False
    for m in (0, 1):
        sfx = str(m + 1)
        mp[f"wb{m}"] = wmat(params["bridge" + sfx]["w"], H)
        mp[f"bb{m}"] = bcol(params["bridge" + sfx]["b"])
        mp[f"wvs{m}"] = wmat(params["sa" + sfx]["wv"], H)
        mp[f"bvs{m}"] = bcol(params["sa" + sfx]["bv"])
        mp[f"wvc{m}"] = wmat(params["ca" + sfx]["wv"], H)
        mp[f"bvc{m}"] = bcol(params["ca" + sfx]["bv"])
        mp[f"w1{m}"] = wmat(params["ffn" + sfx]["w1"], FFN)
        mp[f"b1{m}"] = bcol(params["ffn" + sfx]["b1"])
        mp[f"w2{m}"] = wmat(params["ffn" + sfx]["w2"], H)
        mp[f"b2b{m}"] = np.ascontiguousarray(
            np.broadcast_to(npf(params["ffn" + sfx]["b2"]), (P, H)))
        la, lb, lc = ln_names[m]
        for lk, a_sfx in ((la, "_a"), (lb, "_b")):
            g = npf(params[lk]["g"]); b = npf(params[lk]["b"])
            if not (np.all(g == 1.0) and np.all(b == 0.0)):
                affine14 = True
            mp[f"g{m}{a_sfx}"] = bcol(g)
            mp[f"b{m}{a_sfx}"] = bcol(b)
        g = npf(params[lc]["g"]); b = npf(params[lc]["b"])
        if not (np.all(g == 1.0) and np.all(b == 0.0)):
            affine56 = True
        mp[f"g5b{m}"] = np.ascontiguousarray(np.broadcast_to(g, (P, H)))
        mp[f"b5b{m}"] = np.ascontiguousarray(np.broadcast_to(b, (P, H)))
    if not affine14:
        for m in (0, 1):
            for k in (f"g{m}_a", f"b{m}_a", f"g{m}_b", f"b{m}_b"):
                del mp[k]
    if not affine56:
        for m in (0, 1):
            del mp[f"g5b{m}"]
            del mp[f"b5b{m}"]
    mp["ident32"] = np.eye(P, dtype=np.float32)
    ones_stat = np.zeros((P, 4), ml_dtypes.bfloat16); ones_stat[:, 0] = 1.0
    mp["ones_stat"] = ones_stat
    mp["onesrow_r"] = np.ones((1, P), np.float32)
    mp["eps1"] = np.full((1, 4), EPS, np.float32)
    mp["eps128"] = np.full((P, 4), EPS, np.float32)
    return mp, affine14, affine56


def _get_program(n_tok, affine14, affine56):
    key = (n_tok, affine14, affine56)
    if key not in _CACHE:
        _CACHE[key] = _build(n_tok, affine14, affine56)
    return _CACHE[key]


def make_in_maps(modality_1, modality_2, params, n_cores=N_CORES):
    m1 = np.ascontiguousarray(np.asarray(modality_1, dtype=np.float32))
    m2 = np.ascontiguousarray(np.asarray(modality_2, dtype=np.float32))
    n_tok = m1.shape[0] // n_cores
    assert n_tok % T == 0, f"tokens per core ({n_tok}) must be a multiple of {T}"
    mp, affine14, affine56 = _host_prep(params)
    nc = _get_program(n_tok, affine14, affine56)
    in_maps = []
    for c in range(n_cores):
        d = dict(mp)
        d["x1"] = m1[c * n_tok:(c + 1) * n_tok]
        d["x2"] = m2[c * n_tok:(c + 1) * n_tok]
        in_maps.append(d)
    return nc, in_maps


def run(modality_1, modality_2, params, n_cores=N_CORES):
    nc, in_maps = make_in_maps(modality_1, modality_2, params, n_cores)
    res = run_bass_kernel_spmd(nc, in_maps, list(range(n_cores)))
    o1 = np.concatenate([res.results[c]["out1"] for c in range(n_cores)], axis=0)
    o2 = np.concatenate([res.results[c]["out2"] for c in range(n_cores)], axis=0)
    return o1, o2


def kernel(modality_1, modality_2, params):
    return run(modality_1, modality_2, params)
